# revision 1
# baseline (speedup 1.0000x reference)
"""Bass/Tile TRN2 kernel for nn_Attention_12489764897521.

attns[b, n] = sum_h W[0, h] * tanh(decoder[b, h] + static[b, h, n] + dynamic[b, h, n])

Full shapes: static/dynamic [32, 256, 10000] f32, decoder [32, 256] f32,
W [1, 256] f32 -> attns [32, 10000] f32.

Sharding: data-parallel over batch B across 8 cores (4 batches/core); W
replicated. Per core the kernel is HBM-bandwidth-bound (~82 MB of input
at ~358 GB/s/core => ~230 us roofline).

Per-core dataflow, for each (batch b, n-chunk c of 5000):
  - DMA static/dynamic tiles [128, 5000] per H-half (2.56 MB transfers)
  - DVE: s += d
  - ACT: s = tanh(s + decoder_col)   (per-partition bias)
  - PE: psum[1, 500] = W0.T @ s0_chunk  (start) then += W1.T @ s1_chunk (stop)
  - DVE: psum -> sbuf staging row
  - DMA staging [1, 5000] -> attns[b, chunk]
"""

from contextlib import ExitStack

import numpy as np

B, H, N = 32, 256, 10000
N_CORES = 8
B_LOC = B // N_CORES  # 4 batches per core
P = 128
NT = H // P  # 2 H-halves
NC = 5000  # n-chunk width per SBUF tile
N_CHUNKS = N // NC  # 2
JC = 500  # matmul free-dim chunk (<= 512, one PSUM bank)
NJ = NC // JC  # 10

_cache = {}


def _build():
    import concourse.bacc as bacc
    import concourse.mybir as mybir
    import concourse.tile as tile

    nc = bacc.Bacc(
        "TRN2", target_bir_lowering=False, debug=False, num_devices=N_CORES
    )
    st = nc.dram_tensor(
        "static_hidden", [B_LOC, H, N], mybir.dt.float32, kind="ExternalInput"
    ).ap()
    dy = nc.dram_tensor(
        "dynamic_hidden", [B_LOC, H, N], mybir.dt.float32, kind="ExternalInput"
    ).ap()
    dec = nc.dram_tensor(
        "decoder_hidden", [B_LOC, H], mybir.dt.float32, kind="ExternalInput"
    ).ap()
    w = nc.dram_tensor("W", [1, H], mybir.dt.float32, kind="ExternalInput").ap()
    out = nc.dram_tensor(
        "attns", [B_LOC, N], mybir.dt.float32, kind="ExternalOutput"
    ).ap()

    f32 = mybir.dt.float32
    with tile.TileContext(nc) as tc, ExitStack() as ctx:
        singles = ctx.enter_context(tc.tile_pool(name="singles", bufs=1))
        s_pool = ctx.enter_context(tc.tile_pool(name="s", bufs=4))
        d_pool = ctx.enter_context(tc.tile_pool(name="d", bufs=3))
        stage_pool = ctx.enter_context(tc.tile_pool(name="stage", bufs=2))
        psum_pool = ctx.enter_context(
            tc.tile_pool(name="psum", bufs=4, space="PSUM")
        )

        # W as two [128, 1] columns (one per H-half), decoder as [128, 1]
        # bias columns indexed [t * B_LOC + b].
        w_sb = singles.tile([P, NT], f32)
        w_r = w.rearrange("o (t p) -> t p o", p=P)
        for t in range(NT):
            nc.sync.dma_start(w_sb[:, t : t + 1], w_r[t])

        dec_sb = singles.tile([P, NT * B_LOC], f32)
        dec_r = dec.rearrange("b (t p) -> t p b", p=P)
        for t in range(NT):
            nc.sync.dma_start(dec_sb[:, t * B_LOC : (t + 1) * B_LOC], dec_r[t])

        for b in range(B_LOC):
            for c in range(N_CHUNKS):
                n0 = c * NC
                tanh_tiles = []
                for t in range(NT):
                    s_t = s_pool.tile([P, NC], f32)
                    nc.sync.dma_start(s_t[:], st[b, t * P : (t + 1) * P, n0 : n0 + NC])
                    d_t = d_pool.tile([P, NC], f32)
                    nc.sync.dma_start(d_t[:], dy[b, t * P : (t + 1) * P, n0 : n0 + NC])
                    nc.vector.tensor_add(s_t[:], s_t[:], d_t[:])
                    nc.scalar.activation(
                        s_t[:],
                        s_t[:],
                        mybir.ActivationFunctionType.Tanh,
                        bias=dec_sb[:, t * B_LOC + b : t * B_LOC + b + 1],
                    )
                    tanh_tiles.append(s_t)
                stage = stage_pool.tile([1, NC], f32)
                for j in range(NJ):
                    jl = slice(j * JC, (j + 1) * JC)
                    pt = psum_pool.tile([1, JC], f32)
                    nc.tensor.matmul(
                        pt[:1, :], w_sb[:, 0:1], tanh_tiles[0][:, jl],
                        start=True, stop=False,
                    )
                    nc.tensor.matmul(
                        pt[:1, :], w_sb[:, 1:2], tanh_tiles[1][:, jl],
                        start=False, stop=True,
                    )
                    nc.vector.tensor_copy(stage[:, jl], pt[:1, :])
                nc.sync.dma_start(out[b : b + 1, n0 : n0 + NC], stage[:])

    nc.compile()
    return nc


def _run(inputs, **spmd_kwargs):
    from concourse import bass_utils

    if "nc" not in _cache:
        _cache["nc"] = _build()
    nc = _cache["nc"]

    static_hidden = np.asarray(inputs["static_hidden"], dtype=np.float32)
    dynamic_hidden = np.asarray(inputs["dynamic_hidden"], dtype=np.float32)
    decoder_hidden = np.asarray(inputs["decoder_hidden"], dtype=np.float32)
    W = np.ascontiguousarray(np.asarray(inputs["W"], dtype=np.float32))

    in_maps = []
    for i in range(N_CORES):
        sl = slice(i * B_LOC, (i + 1) * B_LOC)
        in_maps.append(
            {
                "static_hidden": np.ascontiguousarray(static_hidden[sl]),
                "dynamic_hidden": np.ascontiguousarray(dynamic_hidden[sl]),
                "decoder_hidden": np.ascontiguousarray(decoder_hidden[sl]),
                "W": W,
            }
        )
    res = bass_utils.run_bass_kernel_spmd(
        nc, in_maps, core_ids=list(range(N_CORES)), **spmd_kwargs
    )
    out = np.concatenate([r["attns"] for r in res.results], axis=0)
    return out, res


def kernel(**inputs):
    out, _ = _run(inputs)
    return out


# revision 6
# speedup vs baseline: 1.1062x; 1.1062x over previous
"""Bass/Tile TRN2 kernel for nn_Attention_12489764897521.

attns[b, n] = sum_h W[0, h] * tanh(decoder[b, h] + static[b, h, n] + dynamic[b, h, n])

Full shapes: static/dynamic [32, 256, 10000] f32, decoder [32, 256] f32,
W [1, 256] f32 -> attns [32, 10000] f32.

Sharding: data-parallel over batch B across 8 cores (4 batches/core); W
replicated. Per core the kernel is HBM-bandwidth-bound (~82 MB of input
at ~358 GB/s/core => ~230 us roofline).

Per-core dataflow, for each (batch b, n-chunk c of 5000):
  - DMA static/dynamic tiles [128, 5000] per H-half (2.56 MB transfers)
  - DVE: s += d
  - ACT: s = tanh(s + decoder_col)   (per-partition bias)
  - PE: psum[1, 500] = W0.T @ s0_chunk  (start) then += W1.T @ s1_chunk (stop)
  - DVE: psum -> sbuf staging row
  - DMA staging [1, 5000] -> attns[b, chunk]
"""

from contextlib import ExitStack

import numpy as np

B, H, N = 32, 256, 10000
N_CORES = 8
B_LOC = B // N_CORES  # 4 batches per core
P = 128
NT = H // P  # 2 H-halves
NC = 5000  # n-chunk width per SBUF tile
N_CHUNKS = N // NC  # 2
JC = 500  # matmul free-dim chunk (<= 512, one PSUM bank)
NJ = NC // JC  # 10

_cache = {}


def _build():
    import concourse.bacc as bacc
    import concourse.mybir as mybir
    import concourse.tile as tile

    nc = bacc.Bacc(
        "TRN2", target_bir_lowering=False, debug=False, num_devices=N_CORES
    )
    st = nc.dram_tensor(
        "static_hidden", [B_LOC, H, N], mybir.dt.float32, kind="ExternalInput"
    ).ap()
    dy = nc.dram_tensor(
        "dynamic_hidden", [B_LOC, H, N], mybir.dt.float32, kind="ExternalInput"
    ).ap()
    dec = nc.dram_tensor(
        "decoder_hidden", [B_LOC, H], mybir.dt.float32, kind="ExternalInput"
    ).ap()
    w = nc.dram_tensor("W", [1, H], mybir.dt.float32, kind="ExternalInput").ap()
    out = nc.dram_tensor(
        "attns", [B_LOC, N], mybir.dt.float32, kind="ExternalOutput"
    ).ap()

    f32 = mybir.dt.float32
    with tile.TileContext(nc) as tc, ExitStack() as ctx:
        singles = ctx.enter_context(tc.tile_pool(name="singles", bufs=1))
        s_pool = ctx.enter_context(tc.tile_pool(name="s", bufs=2))
        d_pool = ctx.enter_context(tc.tile_pool(name="d", bufs=2))
        t_pool = ctx.enter_context(tc.tile_pool(name="t", bufs=3))
        stage_pool = ctx.enter_context(tc.tile_pool(name="stage", bufs=2))
        psum_pool = ctx.enter_context(
            tc.tile_pool(name="psum", bufs=4, space="PSUM")
        )

        # W as two [128, 1] columns (one per H-half), decoder as [128, 1]
        # bias columns indexed [t * B_LOC + b].
        w_sb = singles.tile([P, NT], f32)
        w_r = w.rearrange("o (t p) -> t p o", p=P)
        for t in range(NT):
            nc.sync.dma_start(w_sb[:, t : t + 1], w_r[t])

        dec_sb = singles.tile([P, NT * B_LOC], f32)
        dec_r = dec.rearrange("b (t p) -> t p b", p=P)
        for t in range(NT):
            nc.sync.dma_start(dec_sb[:, t * B_LOC : (t + 1) * B_LOC], dec_r[t])

        # fp32r inputs run the PE at 1 cycle/row (vs 4 for fp32) once the
        # moving free dim is >= 256. The BIR verifier requires fp32r matmul
        # inputs to be *produced* fp32r-rounded, so tanh writes float32r
        # tiles and W gets a tiny rounding copy on DVE.
        f32r = mybir.dt.float32r
        w_r = singles.tile([P, NT], f32r)
        nc.vector.tensor_copy(w_r[:], w_sb[:])

        for b in range(B_LOC):
            for c in range(N_CHUNKS):
                n0 = c * NC
                tanh_tiles = []
                for t in range(NT):
                    s_t = s_pool.tile([P, NC], f32)
                    nc.sync.dma_start(s_t[:], st[b, t * P : (t + 1) * P, n0 : n0 + NC])
                    d_t = d_pool.tile([P, NC], f32)
                    nc.sync.dma_start(d_t[:], dy[b, t * P : (t + 1) * P, n0 : n0 + NC])
                    nc.vector.tensor_add(s_t[:], s_t[:], d_t[:])
                    t_t = t_pool.tile([P, NC], f32r)
                    nc.scalar.activation(
                        t_t[:],
                        s_t[:],
                        mybir.ActivationFunctionType.Tanh,
                        bias=dec_sb[:, t * B_LOC + b : t * B_LOC + b + 1],
                    )
                    tanh_tiles.append(t_t)
                stage = stage_pool.tile([1, NC], f32)
                for j in range(NJ):
                    jl = slice(j * JC, (j + 1) * JC)
                    pt = psum_pool.tile([1, JC], f32)
                    nc.tensor.matmul(
                        pt[:1, :], w_r[:, 0:1], tanh_tiles[0][:, jl],
                        start=True, stop=False,
                    )
                    nc.tensor.matmul(
                        pt[:1, :], w_r[:, 1:2], tanh_tiles[1][:, jl],
                        start=False, stop=True,
                    )
                    # Single-lane PSUM->SBUF copies; alternate engines so
                    # neither DVE nor ACT eats the whole cost.
                    if j % 2 == 0:
                        nc.vector.tensor_copy(stage[:, jl], pt[:1, :])
                    else:
                        nc.scalar.copy(stage[:, jl], pt[:1, :])
                nc.sync.dma_start(out[b : b + 1, n0 : n0 + NC], stage[:])

    nc.compile()
    return nc


def _run(inputs, **spmd_kwargs):
    from concourse import bass_utils

    if "nc" not in _cache:
        _cache["nc"] = _build()
    nc = _cache["nc"]

    static_hidden = np.asarray(inputs["static_hidden"], dtype=np.float32)
    dynamic_hidden = np.asarray(inputs["dynamic_hidden"], dtype=np.float32)
    decoder_hidden = np.asarray(inputs["decoder_hidden"], dtype=np.float32)
    W = np.ascontiguousarray(np.asarray(inputs["W"], dtype=np.float32))

    in_maps = []
    for i in range(N_CORES):
        sl = slice(i * B_LOC, (i + 1) * B_LOC)
        in_maps.append(
            {
                "static_hidden": np.ascontiguousarray(static_hidden[sl]),
                "dynamic_hidden": np.ascontiguousarray(dynamic_hidden[sl]),
                "decoder_hidden": np.ascontiguousarray(decoder_hidden[sl]),
                "W": W,
            }
        )
    res = bass_utils.run_bass_kernel_spmd(
        nc, in_maps, core_ids=list(range(N_CORES)), **spmd_kwargs
    )
    out = np.concatenate([r["attns"] for r in res.results], axis=0)
    return out, res


def kernel(**inputs):
    out, _ = _run(inputs)
    return out


# revision 8
# speedup vs baseline: 1.1610x; 1.0496x over previous
"""Bass/Tile TRN2 kernel for nn_Attention_12489764897521.

attns[b, n] = sum_h W[0, h] * tanh(decoder[b, h] + static[b, h, n] + dynamic[b, h, n])

Full shapes: static/dynamic [32, 256, 10000] f32, decoder [32, 256] f32,
W [1, 256] f32 -> attns [32, 10000] f32.

Sharding: data-parallel over batch B across 8 cores (4 batches/core); W
replicated. Per core the kernel is HBM-bandwidth-bound (~82 MB of input
at ~358 GB/s/core => ~230 us roofline).

Per-core dataflow, for each (batch b, n-chunk c of 5000):
  - DMA static/dynamic tiles [128, 5000] per H-half (2.56 MB transfers)
  - DVE: s += d
  - ACT: s = tanh(s + decoder_col)   (per-partition bias)
  - PE: psum[1, 500] = W0.T @ s0_chunk  (start) then += W1.T @ s1_chunk (stop)
  - DVE: psum -> sbuf staging row
  - DMA staging [1, 5000] -> attns[b, chunk]
"""

from contextlib import ExitStack

import numpy as np

B, H, N = 32, 256, 10000
N_CORES = 8
B_LOC = B // N_CORES  # 4 batches per core
P = 128
NT = H // P  # 2 H-halves
NC = 5000  # n-chunk width per SBUF tile
N_CHUNKS = N // NC  # 2
JC = 500  # matmul free-dim chunk (<= 512, one PSUM bank)
NJ = NC // JC  # 10

_cache = {}


def _build():
    import concourse.bacc as bacc
    import concourse.mybir as mybir
    import concourse.tile as tile

    nc = bacc.Bacc(
        "TRN2", target_bir_lowering=False, debug=False, num_devices=N_CORES
    )
    st = nc.dram_tensor(
        "static_hidden", [B_LOC, H, N], mybir.dt.float32, kind="ExternalInput"
    ).ap()
    dy = nc.dram_tensor(
        "dynamic_hidden", [B_LOC, H, N], mybir.dt.float32, kind="ExternalInput"
    ).ap()
    dec = nc.dram_tensor(
        "decoder_hidden", [B_LOC, H], mybir.dt.float32, kind="ExternalInput"
    ).ap()
    w = nc.dram_tensor("W", [1, H], mybir.dt.float32, kind="ExternalInput").ap()
    out = nc.dram_tensor(
        "attns", [B_LOC, N], mybir.dt.float32, kind="ExternalOutput"
    ).ap()

    f32 = mybir.dt.float32
    with tile.TileContext(nc) as tc, ExitStack() as ctx:
        singles = ctx.enter_context(tc.tile_pool(name="singles", bufs=1))
        s_pool = ctx.enter_context(tc.tile_pool(name="s", bufs=3))
        d_pool = ctx.enter_context(tc.tile_pool(name="d", bufs=2))
        t_pool = ctx.enter_context(tc.tile_pool(name="t", bufs=3))
        stage_pool = ctx.enter_context(tc.tile_pool(name="stage", bufs=1))
        psum_pool = ctx.enter_context(
            tc.tile_pool(name="psum", bufs=8, space="PSUM")
        )

        # W as two [128, 1] columns (one per H-half), decoder as [128, 1]
        # bias columns indexed [t * B_LOC + b].
        w_sb = singles.tile([P, NT], f32)
        w_r = w.rearrange("o (t p) -> t p o", p=P)
        for t in range(NT):
            nc.sync.dma_start(w_sb[:, t : t + 1], w_r[t])

        dec_sb = singles.tile([P, NT * B_LOC], f32)
        dec_r = dec.rearrange("b (t p) -> t p b", p=P)
        for t in range(NT):
            nc.sync.dma_start(dec_sb[:, t * B_LOC : (t + 1) * B_LOC], dec_r[t])

        # fp32r inputs run the PE at 1 cycle/row (vs 4 for fp32) once the
        # moving free dim is >= 256. The BIR verifier requires fp32r matmul
        # inputs to be *produced* fp32r-rounded, so tanh writes float32r
        # tiles and W gets a tiny rounding copy on DVE.
        f32r = mybir.dt.float32r
        w_r = singles.tile([P, NT], f32r)
        nc.vector.tensor_copy(w_r[:], w_sb[:])

        # The very last chunks are tapered so the trailing serial chain
        # (add -> tanh -> matmul -> copy -> store) after the final load is
        # short; everything else uses full NC-wide chunks.
        def chunks_for(b):
            if b < B_LOC - 1:
                return [(c * NC, NC) for c in range(N_CHUNKS)]
            return [(0, 5000), (5000, 2500), (7500, 1500), (9000, 1000)]

        for b in range(B_LOC):
            for n0, ncw in chunks_for(b):
                tanh_tiles = []
                for t in range(NT):
                    s_t = s_pool.tile([P, ncw], f32, tag="s")
                    nc.sync.dma_start(s_t[:], st[b, t * P : (t + 1) * P, n0 : n0 + ncw])
                    d_t = d_pool.tile([P, ncw], f32, tag="d")
                    nc.sync.dma_start(d_t[:], dy[b, t * P : (t + 1) * P, n0 : n0 + ncw])
                    nc.vector.tensor_add(s_t[:], s_t[:], d_t[:])
                    t_t = t_pool.tile([P, ncw], f32r, tag="t")
                    nc.scalar.activation(
                        t_t[:],
                        s_t[:],
                        mybir.ActivationFunctionType.Tanh,
                        bias=dec_sb[:, t * B_LOC + b : t * B_LOC + b + 1],
                    )
                    tanh_tiles.append(t_t)
                stage = stage_pool.tile([1, ncw], f32, tag="stage")
                for j in range(ncw // JC):
                    jl = slice(j * JC, (j + 1) * JC)
                    pt = psum_pool.tile([1, JC], f32, tag="pt")
                    nc.tensor.matmul(
                        pt[:1, :], w_r[:, 0:1], tanh_tiles[0][:, jl],
                        start=True, stop=False,
                    )
                    nc.tensor.matmul(
                        pt[:1, :], w_r[:, 1:2], tanh_tiles[1][:, jl],
                        start=False, stop=True,
                    )
                    # Single-lane PSUM->SBUF copies; alternate engines so
                    # neither DVE nor ACT eats the whole cost.
                    if j % 2 == 0:
                        nc.vector.tensor_copy(stage[:, jl], pt[:1, :])
                    else:
                        nc.scalar.copy(stage[:, jl], pt[:1, :])
                nc.sync.dma_start(out[b : b + 1, n0 : n0 + ncw], stage[:])

    nc.compile()
    return nc


def _run(inputs, **spmd_kwargs):
    from concourse import bass_utils

    if "nc" not in _cache:
        _cache["nc"] = _build()
    nc = _cache["nc"]

    static_hidden = np.asarray(inputs["static_hidden"], dtype=np.float32)
    dynamic_hidden = np.asarray(inputs["dynamic_hidden"], dtype=np.float32)
    decoder_hidden = np.asarray(inputs["decoder_hidden"], dtype=np.float32)
    W = np.ascontiguousarray(np.asarray(inputs["W"], dtype=np.float32))

    in_maps = []
    for i in range(N_CORES):
        sl = slice(i * B_LOC, (i + 1) * B_LOC)
        in_maps.append(
            {
                "static_hidden": np.ascontiguousarray(static_hidden[sl]),
                "dynamic_hidden": np.ascontiguousarray(dynamic_hidden[sl]),
                "decoder_hidden": np.ascontiguousarray(decoder_hidden[sl]),
                "W": W,
            }
        )
    res = bass_utils.run_bass_kernel_spmd(
        nc, in_maps, core_ids=list(range(N_CORES)), **spmd_kwargs
    )
    out = np.concatenate([r["attns"] for r in res.results], axis=0)
    return out, res


def kernel(**inputs):
    out, _ = _run(inputs)
    return out


# revision 9
# speedup vs baseline: 1.1849x; 1.0206x over previous
"""Bass/Tile TRN2 kernel for nn_Attention_12489764897521.

attns[b, n] = sum_h W[0, h] * tanh(decoder[b, h] + static[b, h, n] + dynamic[b, h, n])

Full shapes: static/dynamic [32, 256, 10000] f32, decoder [32, 256] f32,
W [1, 256] f32 -> attns [32, 10000] f32.

Sharding: data-parallel over batch B across 8 cores (4 batches/core); W
replicated. Per core the kernel is HBM-bandwidth-bound (~82 MB of input
at ~358 GB/s/core => ~230 us roofline).

Per-core dataflow, for each (batch b, n-chunk c of 5000):
  - DMA static/dynamic tiles [128, 5000] per H-half (2.56 MB transfers)
  - DVE: s += d
  - ACT: s = tanh(s + decoder_col)   (per-partition bias)
  - PE: psum[1, 500] = W0.T @ s0_chunk  (start) then += W1.T @ s1_chunk (stop)
  - DVE: psum -> sbuf staging row
  - DMA staging [1, 5000] -> attns[b, chunk]
"""

from contextlib import ExitStack

import numpy as np

B, H, N = 32, 256, 10000
N_CORES = 8
B_LOC = B // N_CORES  # 4 batches per core
P = 128
NT = H // P  # 2 H-halves
NC = 5000  # n-chunk width per SBUF tile
N_CHUNKS = N // NC  # 2
JC = 500  # matmul free-dim chunk (<= 512, one PSUM bank)
NJ = NC // JC  # 10

_cache = {}


def _build():
    import concourse.bacc as bacc
    import concourse.mybir as mybir
    import concourse.tile as tile

    nc = bacc.Bacc(
        "TRN2", target_bir_lowering=False, debug=False, num_devices=N_CORES
    )
    st = nc.dram_tensor(
        "static_hidden", [B_LOC, H, N], mybir.dt.float32, kind="ExternalInput"
    ).ap()
    dy = nc.dram_tensor(
        "dynamic_hidden", [B_LOC, H, N], mybir.dt.float32, kind="ExternalInput"
    ).ap()
    dec = nc.dram_tensor(
        "decoder_hidden", [B_LOC, H], mybir.dt.float32, kind="ExternalInput"
    ).ap()
    w = nc.dram_tensor("W", [1, H], mybir.dt.float32, kind="ExternalInput").ap()
    out = nc.dram_tensor(
        "attns", [B_LOC, N], mybir.dt.float32, kind="ExternalOutput"
    ).ap()

    f32 = mybir.dt.float32
    with tile.TileContext(nc) as tc, ExitStack() as ctx:
        singles = ctx.enter_context(tc.tile_pool(name="singles", bufs=1))
        s_pool = ctx.enter_context(tc.tile_pool(name="s", bufs=3))
        d_pool = ctx.enter_context(tc.tile_pool(name="d", bufs=2))
        t_pool = ctx.enter_context(tc.tile_pool(name="t", bufs=3))
        stage_pool = ctx.enter_context(tc.tile_pool(name="stage", bufs=1))
        psum_pool = ctx.enter_context(
            tc.tile_pool(name="psum", bufs=8, space="PSUM")
        )

        # W as two [128, 1] columns (one per H-half), decoder as [128, 1]
        # bias columns indexed [t * B_LOC + b].
        w_sb = singles.tile([P, NT], f32)
        w_r = w.rearrange("o (t p) -> t p o", p=P)
        for t in range(NT):
            nc.sync.dma_start(w_sb[:, t : t + 1], w_r[t])

        dec_sb = singles.tile([P, NT * B_LOC], f32)
        dec_r = dec.rearrange("b (t p) -> t p b", p=P)
        for t in range(NT):
            nc.sync.dma_start(dec_sb[:, t * B_LOC : (t + 1) * B_LOC], dec_r[t])

        # fp32r inputs run the PE at 1 cycle/row (vs 4 for fp32) once the
        # moving free dim is >= 256. The BIR verifier requires fp32r matmul
        # inputs to be *produced* fp32r-rounded, so tanh writes float32r
        # tiles and W gets a tiny rounding copy on DVE.
        f32r = mybir.dt.float32r
        w_r = singles.tile([P, NT], f32r)
        nc.vector.tensor_copy(w_r[:], w_sb[:])

        # The very last chunks are tapered so the trailing serial chain
        # (add -> tanh -> matmul -> copy -> store) after the final load is
        # short; everything else uses full NC-wide chunks.
        def chunks_for(b):
            if b < B_LOC - 1:
                return [(c * NC, NC) for c in range(N_CHUNKS)]
            return [(0, 5000), (5000, 2500), (7500, 1500), (9000, 1000)]

        for b in range(B_LOC):
            for n0, ncw in chunks_for(b):
                tanh_tiles = []
                for t in range(NT):
                    s_t = s_pool.tile([P, ncw], f32, tag="s")
                    nc.sync.dma_start(s_t[:], st[b, t * P : (t + 1) * P, n0 : n0 + ncw])
                    d_t = d_pool.tile([P, ncw], f32, tag="d")
                    # Issue dynamic loads on the ACT HWDGE ring (qActDynamicHW)
                    # so the two load streams use both hardware DGE rings.
                    nc.scalar.dma_start(d_t[:], dy[b, t * P : (t + 1) * P, n0 : n0 + ncw])
                    nc.vector.tensor_add(s_t[:], s_t[:], d_t[:])
                    t_t = t_pool.tile([P, ncw], f32r, tag="t")
                    nc.scalar.activation(
                        t_t[:],
                        s_t[:],
                        mybir.ActivationFunctionType.Tanh,
                        bias=dec_sb[:, t * B_LOC + b : t * B_LOC + b + 1],
                    )
                    tanh_tiles.append(t_t)
                stage = stage_pool.tile([1, ncw], f32, tag="stage")
                for j in range(ncw // JC):
                    jl = slice(j * JC, (j + 1) * JC)
                    pt = psum_pool.tile([1, JC], f32, tag="pt")
                    nc.tensor.matmul(
                        pt[:1, :], w_r[:, 0:1], tanh_tiles[0][:, jl],
                        start=True, stop=False,
                    )
                    nc.tensor.matmul(
                        pt[:1, :], w_r[:, 1:2], tanh_tiles[1][:, jl],
                        start=False, stop=True,
                    )
                    # Single-lane PSUM->SBUF copies; alternate engines so
                    # neither DVE nor ACT eats the whole cost.
                    if j % 2 == 0:
                        nc.vector.tensor_copy(stage[:, jl], pt[:1, :])
                    else:
                        nc.scalar.copy(stage[:, jl], pt[:1, :])
                nc.sync.dma_start(out[b : b + 1, n0 : n0 + ncw], stage[:])

    nc.compile()
    return nc


def _run(inputs, **spmd_kwargs):
    from concourse import bass_utils

    if "nc" not in _cache:
        _cache["nc"] = _build()
    nc = _cache["nc"]

    static_hidden = np.asarray(inputs["static_hidden"], dtype=np.float32)
    dynamic_hidden = np.asarray(inputs["dynamic_hidden"], dtype=np.float32)
    decoder_hidden = np.asarray(inputs["decoder_hidden"], dtype=np.float32)
    W = np.ascontiguousarray(np.asarray(inputs["W"], dtype=np.float32))

    in_maps = []
    for i in range(N_CORES):
        sl = slice(i * B_LOC, (i + 1) * B_LOC)
        in_maps.append(
            {
                "static_hidden": np.ascontiguousarray(static_hidden[sl]),
                "dynamic_hidden": np.ascontiguousarray(dynamic_hidden[sl]),
                "decoder_hidden": np.ascontiguousarray(decoder_hidden[sl]),
                "W": W,
            }
        )
    res = bass_utils.run_bass_kernel_spmd(
        nc, in_maps, core_ids=list(range(N_CORES)), **spmd_kwargs
    )
    out = np.concatenate([r["attns"] for r in res.results], axis=0)
    return out, res


def kernel(**inputs):
    out, _ = _run(inputs)
    return out
